# revision 13
# baseline (speedup 1.0000x reference)
"""CenterLoss Trainium2 kernel (hybrid gather: PE matmul || SWDGE indirect).

loss = mean_b clip(||x_b - centers[labels_b]||^2, 1e-12, 1e12)
x [8192, 512] f32, labels [8192] int64 in [0, 10000), centers [10000, 512]
f32 -> f32 scalar.  Data-parallel over batch across 8 cores; all bulk data
is cast to fp8_e4m3 on the host (~8e-4 rel err against the 2e-2 budget).

The serial bottleneck of a pure indirect-DMA design is Q7 descriptor
generation: ~1.4 us per 128-row gather instruction, ~11 us for 1024 rows.
This kernel splits the gather across two independent engines:

  tiles 0-3: PE one-hot matmul (PSUM[t] = sel_lo.T @ slab_lo +
             sel_hi.T @ slab_hi), fed by dense HWDGE slab loads
  tiles 4-7: SWDGE indirect DMAs (labels as row offsets), as before

Host side: sort all 8192 rows by label, cut into 8 cores x 1024 rows
(row t*128+p -> partition p, tile t).  For the PE tiles the 128 labels
of a tile span < 256 consecutive center rows, so the host ships two
128-row slab windows + a one-hot selection matrix per tile.  The SWDGE
tiles get plain int32 labels.

Compute: DVE subtracts (PSUM for tiles 0-3, SBUF cbig for 4-7), squares
+ row-reduce on ACT for tiles {0,1,2,3,6,7} and on DVE for {4,5},
interleaved so ACT starts as soon as the first PE tile lands.
"""

import sys

import numpy as np

try:
    import concourse  # noqa: F401
except ImportError:  # pragma: no cover
    sys.path.insert(0, "/opt/trn_rl_repo")

import ml_dtypes

B, D, C = 8192, 512, 10000
N_CORES = 8
P = 128
ROWS = B // N_CORES
T = ROWS // P        # 8 tiles
NPE = 4              # tiles 0..NPE-1 via PE matmul, rest via SWDGE
NCH = 2 * NPE        # slab chunks
WIN = 2 * P
NSLG = 4             # slab DMA groups
ACT_TILES = (0, 1, 2, 3, 4, 5, 6)
# DVE subtracts the PE tiles (PSUM source); GpSimd subtracts the gather
# tiles (its gathers are done by then, and DVE is the saturated engine).
# s_subv ticks per DVE sub (tile order 0..3), s_subg per GpSimd sub (4..7).

CLAMP_MIN = 1e-12
CLAMP_MAX = 1e12

_CACHE = {}


def build():
    import concourse.bacc as bacc
    from concourse import bass, mybir

    f32 = mybir.dt.float32
    bf16 = mybir.dt.bfloat16
    f8 = mybir.dt.float8e4
    i32 = mybir.dt.int32

    nc = bacc.Bacc("TRN2", target_bir_lowering=False, num_devices=N_CORES)
    x = nc.dram_tensor("x", [ROWS, D], f8, kind="ExternalInput")
    # pre-transposed on host: labels4[p, i] = label of row (NPE+i)*128+p
    labels4 = nc.dram_tensor("labels4", [P, T - NPE], i32, kind="ExternalInput")
    slab = nc.dram_tensor("slab", [NCH * P, D], f8, kind="ExternalInput")
    sel = nc.dram_tensor("sel", [P, NCH * P], f8, kind="ExternalInput")
    centers = nc.dram_tensor("centers", [C, D], f8, kind="ExternalInput")
    out = nc.dram_tensor("out", [P, T], f32, kind="ExternalOutput")

    from contextlib import ExitStack

    with (
        nc.Block() as block,
        nc.sbuf_tensor("idx", [P, T - NPE], i32) as idx,
        nc.sbuf_tensor("xbig", [P, T * D], f8) as xbig,
        nc.sbuf_tensor("cbig", [P, (T - NPE) * D], f8) as cbig,
        nc.sbuf_tensor("slabbuf", [P, NCH * D], f8) as slabbuf,
        nc.sbuf_tensor("selbuf", [P, NCH * P], f8) as selbuf,
        nc.sbuf_tensor("diffb", [P, T * D], bf16) as diffb,
        nc.sbuf_tensor("sqb", [P, T * D], bf16) as sqb,
        nc.sbuf_tensor("dist", [P, T], f32) as dist,
        nc.psum_tensor("psum", [P, NPE * D], f32) as psum,
        nc.semaphore("s_idx") as s_idx,
        nc.semaphore("s_sel") as s_sel,
        nc.semaphore("s_pe") as s_pe,
        nc.semaphore("s_subv") as s_subv,
        nc.semaphore("s_subg") as s_subg,
        nc.semaphore("s_m") as s_m,
        nc.semaphore("s_dist") as s_dist,
        nc.semaphore("s_out") as s_out,
        ExitStack() as stack,
    ):
        s_x = [stack.enter_context(nc.semaphore(f"s_x{k}")) for k in range(2)]  # noqa: ANT232
        s_sl = [stack.enter_context(nc.semaphore(f"s_sl{g}")) for g in range(NSLG)]  # noqa: ANT232
        s_g = [stack.enter_context(nc.semaphore(f"s_g{t}")) for t in range(T - NPE)]  # noqa: ANT232

        xsrc = x[:, :].rearrange("(t p) d -> p t d", p=P)
        lsrc = labels4[:, :]
        slsrc = slab[:, :].rearrange("(c p) d -> p c d", p=P)
        CPG = NCH // NSLG

        @block.sync
        def _(sp):
            sp.dma_start(idx[:, :], lsrc).then_inc(s_idx, 16)
            sp.dma_start(selbuf[:, :], sel[:, :]).then_inc(s_sel, 16)
            for g in range(NSLG):
                sl_ = slice(g * CPG * D, (g + 1) * CPG * D)
                sp.dma_start(
                    slabbuf[:, sl_], slsrc[:, g * CPG : (g + 1) * CPG, :]
                ).then_inc(s_sl[g], 16)
            sp.wait_ge(s_dist, T)
            sp.dma_start(out[:, :], dist[:, :]).then_inc(s_out, 16)
            sp.wait_ge(s_out, 16)

        @block.gpsimd
        def _(gp):
            gp.wait_ge(s_idx, 16)
            for i in range(T - NPE):
                gp.indirect_dma_start(
                    out=cbig[:, i * D : (i + 1) * D],
                    out_offset=None,
                    in_=centers[:, :],
                    in_offset=bass.IndirectOffsetOnAxis(ap=idx[:, i : i + 1], axis=0),
                ).then_inc(s_g[i], 16)
            gp.wait_ge(s_x[1], 16)
            for i in range(2):
                t = NPE + i
                sl_ = slice(t * D, (t + 1) * D)
                gp.wait_ge(s_g[i], 16)
                gp.tensor_sub(
                    diffb[:, sl_], xbig[:, sl_], cbig[:, i * D : (i + 1) * D]
                ).then_inc(s_subg, 1)

        @block.tensor
        def _(pe):
            pe.wait_ge(s_sel, 16)
            for t in range(NPE):
                psl = slice(t * D, (t + 1) * D)
                pe.wait_ge(s_sl[(2 * t) // CPG], 16)
                pe.wait_ge(s_sl[(2 * t + 1) // CPG], 16)
                for h in range(2):
                    c = 2 * t + h
                    mm = pe.matmul(
                        out=psum[:, psl],
                        lhsT=selbuf[:, c * P : (c + 1) * P],
                        rhs=slabbuf[:, c * D : (c + 1) * D],
                        start=(h == 0),
                        stop=(h == 1),
                    )
                mm.then_inc(s_pe, 1)

        @block.scalar
        def _(act):
            for k in range(2):
                sl_ = slice(k * (T // 2) * D, (k + 1) * (T // 2) * D)
                act.dma_start(
                    xbig[:, sl_], xsrc[:, k * (T // 2) : (k + 1) * (T // 2), :]
                ).then_inc(s_x[k], 16)
            for t in ACT_TILES:
                if t < NPE:
                    act.wait_ge(s_subv, t + 1)
                elif t in (4, 5):
                    act.wait_ge(s_subg, t - NPE + 1)
                else:  # tile 6: 5th DVE sub
                    act.wait_ge(s_subv, 5)
                sl_ = slice(t * D, (t + 1) * D)
                act.activation(
                    sqb[:, sl_],
                    diffb[:, sl_],
                    mybir.ActivationFunctionType.Square,
                    accum_out=dist[:, t : t + 1],
                ).then_inc(s_dist, 1)

        @block.vector
        def _(v):
            def sub_psum(t):
                sl_ = slice(t * D, (t + 1) * D)
                v.wait_ge(s_x[t // (T // 2)], 16)
                v.wait_ge(s_pe, t + 1)
                v.tensor_sub(diffb[:, sl_], xbig[:, sl_], psum[:, sl_]).then_inc(
                    s_subv, 1
                )

            def sub_sbuf(t):
                sl_ = slice(t * D, (t + 1) * D)
                v.wait_ge(s_x[t // (T // 2)], 16)
                v.wait_ge(s_g[t - NPE], 16)
                v.tensor_sub(
                    diffb[:, sl_], xbig[:, sl_], cbig[:, (t - NPE) * D : (t - NPE + 1) * D]
                ).then_inc(s_subv, 1)

            for t in range(NPE):
                sub_psum(t)
            sub_sbuf(6)
            sub_sbuf(7)
            # square+reduce for tile 7 (tile 6 goes to ACT)
            sl_ = slice(7 * D, 8 * D)
            v.wait_ge(s_subv, 6)  # own sub 7 completed
            v.tensor_tensor(
                out=sqb[:, sl_],
                in0=diffb[:, sl_],
                in1=diffb[:, sl_],
                op=mybir.AluOpType.mult,
            ).then_inc(s_m, 1)
            v.wait_ge(s_m, 1)
            v.tensor_reduce(
                out=dist[:, 7:8],
                in_=sqb[:, sl_].rearrange("p (o d) -> p o d", o=1),
                axis=mybir.AxisListType.X,
                op=mybir.AluOpType.add,
            ).then_inc(s_dist, 1)

    nc.compile()
    return nc


def get_nc():
    nc = _CACHE.get("nc")
    if nc is None:
        nc = _CACHE["nc"] = build()
    return nc


def make_in_maps(x, labels, centers):
    labels = np.asarray(labels).astype(np.int64)
    x16 = np.asarray(x).astype(ml_dtypes.float8_e4m3)
    c16 = np.ascontiguousarray(np.asarray(centers).astype(ml_dtypes.float8_e4m3))

    order = np.argsort(labels, kind="stable")
    in_maps = []
    for i in range(N_CORES):
        rows = order[i * ROWS : (i + 1) * ROWS]
        lab = labels[rows]
        sel = np.zeros((P, NCH * P), dtype=ml_dtypes.float8_e4m3)
        slab_idx = np.zeros(NCH * P, dtype=np.int64)
        for t in range(NPE):
            tl = lab[t * P : (t + 1) * P]
            lo = int(tl[0])
            w = tl - lo
            assert w.max() < WIN, "label window exceeds 256; PE tiles invalid"
            h = w // P
            sel[w % P, (2 * t + h) * P + np.arange(P)] = 1.0
            for hh in range(2):
                slab_idx[(2 * t + hh) * P : (2 * t + hh + 1) * P] = np.minimum(
                    lo + hh * P + np.arange(P), C - 1
                )
        in_maps.append(
            {
                "x": np.ascontiguousarray(x16[rows]),
                "labels4": np.ascontiguousarray(
                    lab[NPE * P :].astype(np.int32).reshape(T - NPE, P).T
                ),
                "slab": np.ascontiguousarray(c16[slab_idx]),
                "sel": sel,
                "centers": c16,
            }
        )
    return in_maps


def finish(per_core_outs):
    d = np.concatenate([np.asarray(o).reshape(-1) for o in per_core_outs])
    d = np.clip(d, CLAMP_MIN, CLAMP_MAX)
    return np.asarray(np.mean(d, dtype=np.float64), dtype=np.float32)


def kernel(x, labels, centers):
    from concourse.bass_utils import run_bass_kernel_spmd

    nc = get_nc()
    in_maps = make_in_maps(x, labels, centers)
    res = run_bass_kernel_spmd(nc, in_maps, core_ids=list(range(N_CORES)))
    return finish([r["out"] for r in res.results])
